# revision 49
# baseline (speedup 1.0000x reference)
"""Trainium2 Bass kernel for the attention-mechanism problem.

Math (reference):
    W_s, W_e = W[:SD], W[SD:]
    score[b]    = state_tm1[b] @ W_s + b_bias                 # [B]
    logits[n,b] = emb[n,b] @ W_e + score[b]                   # [N,B]
    alpha       = softmax(logits, axis=0)                     # over N
    out[b,e]    = sum_n alpha[n,b] * emb[n,b,e]               # [B,ED]

Strategy: data-parallel over B across 8 cores (B_local = 8 per core).
Softmax needs no max subtraction here (logits are O(3)), so a single pass
over the embeddings suffices:
    p[n,b]   = exp(logits[n,b])
    acc[b,e] = sum_n p[n,b] emb[n,b,e]     (TensorE, PSUM accumulation)
    Z[b]     = sum_n p[n,b]
    out      = acc / Z

Per-core layout: the emb shard [N, 8, ED] flattens to rows r = n*8 + b.
SBUF tiles are [128 rows, ED] (16 n-values x all 8 b), so each tile is one
fully contiguous 256 KB block of HBM.  Per tile:
    VectorE : l = rowdot(tile, W_e)         (scalar_tensor_tensor w/ accum)
    ScalarE : p = exp(l + score_bias)       (batched over a DMA group)
    ScalarE : lhsT = mask8 * p              ([128,8], column b nonzero iff r%8==b)
    TensorE : acc[8,ED] += lhsT.T @ tile    (PSUM accumulate)
Z comes from rowsums of the stored p-matrix + one tiny matmul with mask8.
"""

import os

import numpy as np

N, B, SD, ED = 2048, 64, 512, 512
NCORES = 8
BL = B // NCORES  # 8 batch entries per core
P = 128  # SBUF partitions
NT = (N * BL) // P  # 128 tiles of [128, ED] per core
GRP = 8  # tiles per DMA group (2 MB per DMA in fp32)
NG = NT // GRP

# Storage/compute precision for the embeddings (the 256MB streaming input):
#   "float16"  (default): half the HBM traffic, rel err ~3e-4 vs the reference
#   "float32r": fp32 storage, TF32-style matmul, rel err ~1.2e-4
#   "float32":  exact (rel err ~2e-6), fp32 matmul runs at 1/4 PE rate
#   "bfloat16": like float16 but rel err ~2e-3 (no reason to prefer it)
COMPUTE_DTYPE = os.environ.get("ATTN_KERNEL_DTYPE", "float16")

# Best-known build configuration per dtype (HW-validated via bench_server.py).
_BUILD_CFG = {
    "float16": dict(taper=True, emb_bufs=6),
    "bfloat16": dict(taper=True, emb_bufs=6),
    "float32r": dict(taper=True, emb_bufs=6),
    "float32": dict(taper=True, emb_bufs=6),
}

_cache: dict = {}
last_result = None  # BassKernelResults of the most recent run (for profiling)


def _build(
    dt_name: str,
    n: int = N,
    grp: int = GRP,
    reps: int = 1,
    emb_bufs: int = 3,
    tmp_bufs: int = 3,
    lh_bufs: int = 6,
    matmul_dt: str | None = None,
    taper: bool = False,
    gps_frac: float = 0.0,
    tt_ts: bool = False,
    act_frac: float = 0.0,
):
    """reps>1 wraps the whole kernel in a device-side For_i loop — used only
    for timing (one RPC amortizes `reps` kernel executions)."""
    import concourse.mybir as mybir
    import concourse.tile as tile
    from concourse import bacc
    from contextlib import nullcontext

    f32 = mybir.dt.float32
    dt = getattr(mybir.dt, dt_name)
    nt = (n * BL) // P
    ng = nt // grp
    assert nt % grp == 0

    nc = bacc.Bacc("TRN2")
    # With matmul_dt=float32r the emb tensor (and its SBUF tiles) are declared
    # float32r so the fp32r matmul's operand chain satisfies the verifier's
    # rounding rule; the bytes are ordinary fp32 and the DVE reads them
    # through a float32 bitcast view.
    emb_dt = dt
    if matmul_dt == "float32r" and dt_name == "float32":
        emb_dt = mybir.dt.float32r
    emb = nc.dram_tensor("emb", [n * BL, ED], emb_dt, kind="ExternalInput")
    web = nc.dram_tensor("web", [P, ED], dt, kind="ExternalInput")
    bias128 = nc.dram_tensor("bias128", [P, 1], f32, kind="ExternalInput")
    mask8 = nc.dram_tensor("mask8", [P, BL], f32, kind="ExternalInput")
    outd = nc.dram_tensor("out", [BL, ED], f32, kind="ExternalOutput")

    # [n*BL, ED] rows -> [P, nt, ED] view (tile t = rows [128t, 128t+128)).
    # Group sizes: uniform `grp`, or tapered (small head for a fast pipeline
    # start, small tail so the post-DMA compute chain is short).
    if taper:
        head = [1, 1, 2, 4]
        tail = [4, 2, 1, 1]
        mid_total = nt - sum(head) - sum(tail)
        assert mid_total % grp == 0
        groups = head + [grp] * (mid_total // grp) + tail
    else:
        groups = [grp] * ng
    assert sum(groups) == nt
    emb_r = emb[:, :].rearrange("(t p) e -> p t e", p=P)

    with tile.TileContext(nc) as tc:
        with (
            tc.tile_pool(name="consts", bufs=1) as consts,
            tc.tile_pool(name="embp", bufs=emb_bufs) as embp,
            tc.tile_pool(name="tmpp", bufs=tmp_bufs) as tmpp,
            tc.tile_pool(name="lgp", bufs=len(groups)) as lgp,
            tc.tile_pool(name="lhp", bufs=lh_bufs) as lhp,
            tc.tile_pool(name="smalls", bufs=1) as smalls,
            tc.tile_pool(name="psum", bufs=1, space="PSUM") as psum,
        ):
            web_s = consts.tile([P, ED], dt)
            nc.sync.dma_start(out=web_s, in_=web[:, :])
            bias_s = consts.tile([P, 1], f32)
            nc.sync.dma_start(out=bias_s, in_=bias128[:, :])
            mask_s = consts.tile([P, BL], f32)
            nc.sync.dma_start(out=mask_s, in_=mask8[:, :])

            pbuf = consts.tile([P, nt], f32)  # all exp-weights, col t = tile t
            acc = psum.tile([BL, ED], f32)  # weighted-sum accumulator

            # Prime the const-tile dependencies on each consuming engine so
            # the hot-loop instructions need at most one sync wait each (the
            # 2-src-2-dst STT encoding only has a single wait slot).
            dummy_v = smalls.tile([P, 1], dt)
            nc.vector.tensor_copy(out=dummy_v, in_=web_s[:, 0:1])
            if gps_frac > 0:
                dummy_g = smalls.tile([P, 1], dt)
                nc.gpsimd.tensor_copy(out=dummy_g, in_=web_s[:, 0:1])
            dummy_s = smalls.tile([P, 1], f32)
            nc.scalar.activation(
                out=dummy_s,
                in_=mask_s[:, 0:1],
                func=mybir.ActivationFunctionType.Identity,
                bias=bias_s,
                scale=1.0,
            )

            rep_ctx = (
                tc.For_i(0, reps, 1, hint_engines=(mybir.EngineType.PE,))
                if reps > 1
                else nullcontext()
            )
            with rep_ctx:
                _kernel_body(
                    nc, tc, mybir, dt, f32, groups, nt,
                    emb_r, web_s, bias_s, mask_s, pbuf, acc,
                    embp, tmpp, lgp, lhp, smalls, psum, outd,
                    matmul_dt, emb_dt, gps_frac, tt_ts, act_frac,
                )

    nc.finalize()
    return nc


def _kernel_body(
    nc, tc, mybir, dt, f32, groups, nt,
    emb_r, web_s, bias_s, mask_s, pbuf, acc,
    embp, tmpp, lgp, lhp, smalls, psum, outd,
    matmul_dt=None, emb_dt=None, gps_frac=0.0, tt_ts=False, act_frac=0.0,
):
    mdt = getattr(mybir.dt, matmul_dt) if matmul_dt else None
    if emb_dt is None:
        emb_dt = dt
    # Columns [0, ecut) of each row-dot go to VectorE, [ecut, ED) to GpSimd.
    ecut = ED if gps_frac <= 0 else max(4, int(ED * (1.0 - gps_frac)) & ~3)
    # For `act_frac` of the tiles in each group, VectorE only does the
    # (2x-mode) multiply and the free-axis reduce runs on the otherwise-idle
    # ScalarE via activation(Copy, accum_out=...). VectorE op count per tile
    # stays 1, so no extra DVE DRAINs (the trap the tt_ts variant hit).
    if True:
        if True:
            t0 = 0
            for gi, s in enumerate(groups):
                gt = embp.tile([P, max(groups), ED], emb_dt)
                nc.sync.dma_start(
                    out=gt[:, :s, :], in_=emb_r[:, t0 : t0 + s, :]
                )

                # Whole-group engine choice keeps each lg tile single-writer
                # (mixing engines within a group would WAW-serialize on lg).
                act_group = act_frac > 0 and (gi % 8) < round(act_frac * 8)
                lg = lgp.tile(
                    [P, s], f32,
                    name="lgA" if act_group else "lg",
                    tag="lgA" if act_group else "lg",
                )
                lg2 = lgp.tile([P, s], f32, name="lg2", tag="lg2") if ecut < ED else None
                for j in range(s):
                    tmp = tmpp.tile([P, ecut], dt, name="tmp", tag="tmp")
                    gt_j = gt[:, j, :]
                    if emb_dt != dt:
                        gt_j = gt_j.bitcast(dt)
                    if act_group:
                        nc.vector.tensor_mul(
                            out=tmp, in0=gt_j[:, :ecut], in1=web_s[:, :ecut]
                        )
                        nc.scalar.activation(
                            out=tmp,
                            in_=tmp,
                            func=mybir.ActivationFunctionType.Copy,
                            bias=0.0,
                            scale=1.0,
                            accum_out=lg[:, j : j + 1],
                        )
                    elif tt_ts:
                        # 2x-mode multiply, then 4x-mode scalar pass that
                        # carries the free-axis accumulation.
                        nc.vector.tensor_mul(
                            out=tmp, in0=gt_j[:, :ecut], in1=web_s[:, :ecut]
                        )
                        junk = tmpp.tile([P, ecut], dt, name="junk", tag="junk")
                        nc.vector.tensor_scalar(
                            out=junk,
                            in0=tmp,
                            scalar1=1.0,
                            scalar2=0.0,
                            op0=mybir.AluOpType.mult,
                            op1=mybir.AluOpType.add,
                            accum_out=lg[:, j : j + 1],
                        )
                    else:
                        nc.vector.scalar_tensor_tensor(
                            out=tmp,
                            in0=gt_j[:, :ecut],
                            scalar=1.0,
                            in1=web_s[:, :ecut],
                            op0=mybir.AluOpType.mult,
                            op1=mybir.AluOpType.mult,
                            accum_out=lg[:, j : j + 1],
                        )
                    if lg2 is not None:
                        tmp2 = tmpp.tile([P, ED - ecut], dt, name="tmp2", tag="tmp2")
                        nc.gpsimd.scalar_tensor_tensor(
                            out=tmp2,
                            in0=gt_j[:, ecut:],
                            scalar=1.0,
                            in1=web_s[:, ecut:],
                            op0=mybir.AluOpType.mult,
                            op1=mybir.AluOpType.mult,
                            accum_out=lg2[:, j : j + 1],
                        )
                if lg2 is not None:
                    nc.vector.tensor_add(out=lg, in0=lg, in1=lg2)
                nc.scalar.activation(
                    out=pbuf[:, t0 : t0 + s],
                    in_=lg,
                    func=mybir.ActivationFunctionType.Exp,
                    bias=bias_s,
                    scale=1.0,
                )
                for j in range(s):
                    t = t0 + j
                    lh = lhp.tile([P, BL], mdt if mdt is not None else dt)
                    nc.scalar.mul(out=lh, in_=mask_s, mul=pbuf[:, t : t + 1])
                    nc.tensor.matmul(
                        acc,
                        lh,
                        gt[:, j, :],
                        start=(t == 0),
                        stop=(t == nt - 1),
                    )
                t0 += s

            rowsum = smalls.tile([P, 1], f32)
            nc.vector.reduce_sum(out=rowsum, in_=pbuf, axis=mybir.AxisListType.X)
            zp = psum.tile([BL, 1], f32)
            nc.tensor.matmul(zp, mask_s, rowsum, start=True, stop=True)
            rz = smalls.tile([BL, 1], f32)
            nc.vector.reciprocal(out=rz, in_=zp)
            outs = smalls.tile([BL, ED], f32)
            nc.vector.tensor_scalar_mul(out=outs, in0=acc, scalar1=rz)
            nc.sync.dma_start(out=outd[:, :], in_=outs)


def _get_nc(dt_name: str):
    if dt_name not in _cache:
        cfg = dict(_BUILD_CFG.get(dt_name, {}))
        if dt_name == "float32r":
            _cache[dt_name] = _build("float32", matmul_dt="float32r", **cfg)
        else:
            _cache[dt_name] = _build(dt_name, **cfg)
    return _cache[dt_name]


def _make_in_maps(inputs):
    """Shard the full inputs into the 8 per-core input maps."""
    state = np.asarray(inputs["state_tm1"], dtype=np.float32)
    emb = np.asarray(inputs["embeddings"], dtype=np.float32)
    Wf = np.asarray(inputs["W"], dtype=np.float32)
    bf = np.asarray(inputs["b"], dtype=np.float32)

    dt_name = COMPUTE_DTYPE
    if dt_name in ("float32", "float32r"):
        np_dt = np.float32
    elif dt_name == "float16":
        np_dt = np.float16
    else:
        import ml_dtypes

        np_dt = ml_dtypes.bfloat16

    W_e = Wf[SD:, 0]
    score = state @ Wf[:SD, 0] + bf[0]  # [B]

    web = np.ascontiguousarray(np.broadcast_to(W_e[None, :], (P, ED))).astype(np_dt)
    mask8 = (np.arange(P)[:, None] % BL == np.arange(BL)[None, :]).astype(np.float32)
    mask8 = np.ascontiguousarray(mask8)

    in_maps = []
    for c in range(NCORES):
        shard = np.ascontiguousarray(emb[:, c * BL : (c + 1) * BL, :]).reshape(
            N * BL, ED
        )
        shard = shard.astype(np_dt) if np_dt is not np.float32 else shard
        bias = np.ascontiguousarray(
            np.tile(score[c * BL : (c + 1) * BL], P // BL)[:, None].astype(np.float32)
        )
        in_maps.append({"emb": shard, "web": web, "bias128": bias, "mask8": mask8})
    return in_maps


def kernel(state_tm1, embeddings, W, b):
    global last_result
    from concourse.bass_utils import run_bass_kernel_spmd

    in_maps = _make_in_maps(
        dict(state_tm1=state_tm1, embeddings=embeddings, W=W, b=b)
    )
    nc = _get_nc(COMPUTE_DTYPE)
    res = run_bass_kernel_spmd(nc, in_maps, core_ids=list(range(NCORES)))
    last_result = res
    out = np.concatenate([r["out"] for r in res.results], axis=0)
    return out


# revision 50
# speedup vs baseline: 1.0001x; 1.0001x over previous
"""Trainium2 Bass kernel for the attention-mechanism problem.

Math (reference):
    W_s, W_e = W[:SD], W[SD:]
    score[b]    = state_tm1[b] @ W_s + b_bias                 # [B]
    logits[n,b] = emb[n,b] @ W_e + score[b]                   # [N,B]
    alpha       = softmax(logits, axis=0)                     # over N
    out[b,e]    = sum_n alpha[n,b] * emb[n,b,e]               # [B,ED]

Strategy: data-parallel over B across 8 cores (B_local = 8 per core).
Softmax needs no max subtraction here (logits are O(3)), so a single pass
over the embeddings suffices:
    p[n,b]   = exp(logits[n,b])
    acc[b,e] = sum_n p[n,b] emb[n,b,e]     (TensorE, PSUM accumulation)
    Z[b]     = sum_n p[n,b]
    out      = acc / Z

Per-core layout: the emb shard [N, 8, ED] flattens to rows r = n*8 + b.
SBUF tiles are [128 rows, ED] (16 n-values x all 8 b), so each tile is one
fully contiguous 256 KB block of HBM.  Per tile:
    VectorE : l = rowdot(tile, W_e)         (scalar_tensor_tensor w/ accum)
    ScalarE : p = exp(l + score_bias)       (batched over a DMA group)
    ScalarE : lhsT = mask8 * p              ([128,8], column b nonzero iff r%8==b)
    TensorE : acc[8,ED] += lhsT.T @ tile    (PSUM accumulate)
Z comes from rowsums of the stored p-matrix + one tiny matmul with mask8.
"""

import os

import numpy as np

N, B, SD, ED = 2048, 64, 512, 512
NCORES = 8
BL = B // NCORES  # 8 batch entries per core
P = 128  # SBUF partitions
NT = (N * BL) // P  # 128 tiles of [128, ED] per core
GRP = 8  # tiles per DMA group (2 MB per DMA in fp32)
NG = NT // GRP

# Storage/compute precision for the embeddings (the 256MB streaming input):
#   "float16"  (default): half the HBM traffic, rel err ~3e-4 vs the reference
#   "float32r": fp32 storage, TF32-style matmul, rel err ~1.2e-4
#   "float32":  exact (rel err ~2e-6), fp32 matmul runs at 1/4 PE rate
#   "bfloat16": like float16 but rel err ~2e-3 (no reason to prefer it)
COMPUTE_DTYPE = os.environ.get("ATTN_KERNEL_DTYPE", "float16")

# Best-known build configuration per dtype (HW-validated via bench_server.py).
_BUILD_CFG = {
    "float16": dict(taper=True, emb_bufs=10, grp=4),
    "bfloat16": dict(taper=True, emb_bufs=10, grp=4),
    "float32r": dict(taper=True, emb_bufs=6),
    "float32": dict(taper=True, emb_bufs=6),
}

_cache: dict = {}
last_result = None  # BassKernelResults of the most recent run (for profiling)


def _build(
    dt_name: str,
    n: int = N,
    grp: int = GRP,
    reps: int = 1,
    emb_bufs: int = 3,
    tmp_bufs: int = 3,
    lh_bufs: int = 6,
    matmul_dt: str | None = None,
    taper: bool = False,
    gps_frac: float = 0.0,
    tt_ts: bool = False,
    act_frac: float = 0.0,
):
    """reps>1 wraps the whole kernel in a device-side For_i loop — used only
    for timing (one RPC amortizes `reps` kernel executions)."""
    import concourse.mybir as mybir
    import concourse.tile as tile
    from concourse import bacc
    from contextlib import nullcontext

    f32 = mybir.dt.float32
    dt = getattr(mybir.dt, dt_name)
    nt = (n * BL) // P
    ng = nt // grp
    assert nt % grp == 0

    nc = bacc.Bacc("TRN2")
    # With matmul_dt=float32r the emb tensor (and its SBUF tiles) are declared
    # float32r so the fp32r matmul's operand chain satisfies the verifier's
    # rounding rule; the bytes are ordinary fp32 and the DVE reads them
    # through a float32 bitcast view.
    emb_dt = dt
    if matmul_dt == "float32r" and dt_name == "float32":
        emb_dt = mybir.dt.float32r
    emb = nc.dram_tensor("emb", [n * BL, ED], emb_dt, kind="ExternalInput")
    web = nc.dram_tensor("web", [P, ED], dt, kind="ExternalInput")
    bias128 = nc.dram_tensor("bias128", [P, 1], f32, kind="ExternalInput")
    mask8 = nc.dram_tensor("mask8", [P, BL], f32, kind="ExternalInput")
    outd = nc.dram_tensor("out", [BL, ED], f32, kind="ExternalOutput")

    # [n*BL, ED] rows -> [P, nt, ED] view (tile t = rows [128t, 128t+128)).
    # Group sizes: uniform `grp`, or tapered (small head for a fast pipeline
    # start, small tail so the post-DMA compute chain is short).
    if taper:
        head = [1, 1, 2, 4]
        tail = [4, 2, 1, 1]
        mid_total = nt - sum(head) - sum(tail)
        assert mid_total % grp == 0
        groups = head + [grp] * (mid_total // grp) + tail
    else:
        groups = [grp] * ng
    assert sum(groups) == nt
    emb_r = emb[:, :].rearrange("(t p) e -> p t e", p=P)

    with tile.TileContext(nc) as tc:
        with (
            tc.tile_pool(name="consts", bufs=1) as consts,
            tc.tile_pool(name="embp", bufs=emb_bufs) as embp,
            tc.tile_pool(name="tmpp", bufs=tmp_bufs) as tmpp,
            tc.tile_pool(name="lgp", bufs=len(groups)) as lgp,
            tc.tile_pool(name="lhp", bufs=lh_bufs) as lhp,
            tc.tile_pool(name="smalls", bufs=1) as smalls,
            tc.tile_pool(name="psum", bufs=1, space="PSUM") as psum,
        ):
            web_s = consts.tile([P, ED], dt)
            nc.sync.dma_start(out=web_s, in_=web[:, :])
            bias_s = consts.tile([P, 1], f32)
            nc.sync.dma_start(out=bias_s, in_=bias128[:, :])
            mask_s = consts.tile([P, BL], f32)
            nc.sync.dma_start(out=mask_s, in_=mask8[:, :])

            pbuf = consts.tile([P, nt], f32)  # all exp-weights, col t = tile t
            acc = psum.tile([BL, ED], f32)  # weighted-sum accumulator

            # Prime the const-tile dependencies on each consuming engine so
            # the hot-loop instructions need at most one sync wait each (the
            # 2-src-2-dst STT encoding only has a single wait slot).
            dummy_v = smalls.tile([P, 1], dt)
            nc.vector.tensor_copy(out=dummy_v, in_=web_s[:, 0:1])
            if gps_frac > 0:
                dummy_g = smalls.tile([P, 1], dt)
                nc.gpsimd.tensor_copy(out=dummy_g, in_=web_s[:, 0:1])
            dummy_s = smalls.tile([P, 1], f32)
            nc.scalar.activation(
                out=dummy_s,
                in_=mask_s[:, 0:1],
                func=mybir.ActivationFunctionType.Identity,
                bias=bias_s,
                scale=1.0,
            )

            rep_ctx = (
                tc.For_i(0, reps, 1, hint_engines=(mybir.EngineType.PE,))
                if reps > 1
                else nullcontext()
            )
            with rep_ctx:
                _kernel_body(
                    nc, tc, mybir, dt, f32, groups, nt,
                    emb_r, web_s, bias_s, mask_s, pbuf, acc,
                    embp, tmpp, lgp, lhp, smalls, psum, outd,
                    matmul_dt, emb_dt, gps_frac, tt_ts, act_frac,
                )

    nc.finalize()
    return nc


def _kernel_body(
    nc, tc, mybir, dt, f32, groups, nt,
    emb_r, web_s, bias_s, mask_s, pbuf, acc,
    embp, tmpp, lgp, lhp, smalls, psum, outd,
    matmul_dt=None, emb_dt=None, gps_frac=0.0, tt_ts=False, act_frac=0.0,
):
    mdt = getattr(mybir.dt, matmul_dt) if matmul_dt else None
    if emb_dt is None:
        emb_dt = dt
    # Columns [0, ecut) of each row-dot go to VectorE, [ecut, ED) to GpSimd.
    ecut = ED if gps_frac <= 0 else max(4, int(ED * (1.0 - gps_frac)) & ~3)
    # For `act_frac` of the tiles in each group, VectorE only does the
    # (2x-mode) multiply and the free-axis reduce runs on the otherwise-idle
    # ScalarE via activation(Copy, accum_out=...). VectorE op count per tile
    # stays 1, so no extra DVE DRAINs (the trap the tt_ts variant hit).
    if True:
        if True:
            t0 = 0
            for gi, s in enumerate(groups):
                gt = embp.tile([P, max(groups), ED], emb_dt)
                nc.sync.dma_start(
                    out=gt[:, :s, :], in_=emb_r[:, t0 : t0 + s, :]
                )

                # Whole-group engine choice keeps each lg tile single-writer
                # (mixing engines within a group would WAW-serialize on lg).
                act_group = act_frac > 0 and (gi % 8) < round(act_frac * 8)
                lg = lgp.tile(
                    [P, s], f32,
                    name="lgA" if act_group else "lg",
                    tag="lgA" if act_group else "lg",
                )
                lg2 = lgp.tile([P, s], f32, name="lg2", tag="lg2") if ecut < ED else None
                for j in range(s):
                    tmp = tmpp.tile([P, ecut], dt, name="tmp", tag="tmp")
                    gt_j = gt[:, j, :]
                    if emb_dt != dt:
                        gt_j = gt_j.bitcast(dt)
                    if act_group:
                        nc.vector.tensor_mul(
                            out=tmp, in0=gt_j[:, :ecut], in1=web_s[:, :ecut]
                        )
                        nc.scalar.activation(
                            out=tmp,
                            in_=tmp,
                            func=mybir.ActivationFunctionType.Copy,
                            bias=0.0,
                            scale=1.0,
                            accum_out=lg[:, j : j + 1],
                        )
                    elif tt_ts:
                        # 2x-mode multiply, then 4x-mode scalar pass that
                        # carries the free-axis accumulation.
                        nc.vector.tensor_mul(
                            out=tmp, in0=gt_j[:, :ecut], in1=web_s[:, :ecut]
                        )
                        junk = tmpp.tile([P, ecut], dt, name="junk", tag="junk")
                        nc.vector.tensor_scalar(
                            out=junk,
                            in0=tmp,
                            scalar1=1.0,
                            scalar2=0.0,
                            op0=mybir.AluOpType.mult,
                            op1=mybir.AluOpType.add,
                            accum_out=lg[:, j : j + 1],
                        )
                    else:
                        nc.vector.scalar_tensor_tensor(
                            out=tmp,
                            in0=gt_j[:, :ecut],
                            scalar=1.0,
                            in1=web_s[:, :ecut],
                            op0=mybir.AluOpType.mult,
                            op1=mybir.AluOpType.mult,
                            accum_out=lg[:, j : j + 1],
                        )
                    if lg2 is not None:
                        tmp2 = tmpp.tile([P, ED - ecut], dt, name="tmp2", tag="tmp2")
                        nc.gpsimd.scalar_tensor_tensor(
                            out=tmp2,
                            in0=gt_j[:, ecut:],
                            scalar=1.0,
                            in1=web_s[:, ecut:],
                            op0=mybir.AluOpType.mult,
                            op1=mybir.AluOpType.mult,
                            accum_out=lg2[:, j : j + 1],
                        )
                if lg2 is not None:
                    nc.vector.tensor_add(out=lg, in0=lg, in1=lg2)
                nc.scalar.activation(
                    out=pbuf[:, t0 : t0 + s],
                    in_=lg,
                    func=mybir.ActivationFunctionType.Exp,
                    bias=bias_s,
                    scale=1.0,
                )
                for j in range(s):
                    t = t0 + j
                    lh = lhp.tile([P, BL], mdt if mdt is not None else dt)
                    nc.scalar.mul(out=lh, in_=mask_s, mul=pbuf[:, t : t + 1])
                    nc.tensor.matmul(
                        acc,
                        lh,
                        gt[:, j, :],
                        start=(t == 0),
                        stop=(t == nt - 1),
                    )
                t0 += s

            rowsum = smalls.tile([P, 1], f32)
            nc.vector.reduce_sum(out=rowsum, in_=pbuf, axis=mybir.AxisListType.X)
            zp = psum.tile([BL, 1], f32)
            nc.tensor.matmul(zp, mask_s, rowsum, start=True, stop=True)
            rz = smalls.tile([BL, 1], f32)
            nc.vector.reciprocal(out=rz, in_=zp)
            outs = smalls.tile([BL, ED], f32)
            nc.vector.tensor_scalar_mul(out=outs, in0=acc, scalar1=rz)
            nc.sync.dma_start(out=outd[:, :], in_=outs)


def _get_nc(dt_name: str):
    if dt_name not in _cache:
        cfg = dict(_BUILD_CFG.get(dt_name, {}))
        if dt_name == "float32r":
            _cache[dt_name] = _build("float32", matmul_dt="float32r", **cfg)
        else:
            _cache[dt_name] = _build(dt_name, **cfg)
    return _cache[dt_name]


def _make_in_maps(inputs):
    """Shard the full inputs into the 8 per-core input maps."""
    state = np.asarray(inputs["state_tm1"], dtype=np.float32)
    emb = np.asarray(inputs["embeddings"], dtype=np.float32)
    Wf = np.asarray(inputs["W"], dtype=np.float32)
    bf = np.asarray(inputs["b"], dtype=np.float32)

    dt_name = COMPUTE_DTYPE
    if dt_name in ("float32", "float32r"):
        np_dt = np.float32
    elif dt_name == "float16":
        np_dt = np.float16
    else:
        import ml_dtypes

        np_dt = ml_dtypes.bfloat16

    W_e = Wf[SD:, 0]
    score = state @ Wf[:SD, 0] + bf[0]  # [B]

    web = np.ascontiguousarray(np.broadcast_to(W_e[None, :], (P, ED))).astype(np_dt)
    mask8 = (np.arange(P)[:, None] % BL == np.arange(BL)[None, :]).astype(np.float32)
    mask8 = np.ascontiguousarray(mask8)

    in_maps = []
    for c in range(NCORES):
        shard = np.ascontiguousarray(emb[:, c * BL : (c + 1) * BL, :]).reshape(
            N * BL, ED
        )
        shard = shard.astype(np_dt) if np_dt is not np.float32 else shard
        bias = np.ascontiguousarray(
            np.tile(score[c * BL : (c + 1) * BL], P // BL)[:, None].astype(np.float32)
        )
        in_maps.append({"emb": shard, "web": web, "bias128": bias, "mask8": mask8})
    return in_maps


def kernel(state_tm1, embeddings, W, b):
    global last_result
    from concourse.bass_utils import run_bass_kernel_spmd

    in_maps = _make_in_maps(
        dict(state_tm1=state_tm1, embeddings=embeddings, W=W, b=b)
    )
    nc = _get_nc(COMPUTE_DTYPE)
    res = run_bass_kernel_spmd(nc, in_maps, core_ids=list(range(NCORES)))
    last_result = res
    out = np.concatenate([r["out"] for r in res.results], axis=0)
    return out
